# revision 23
# baseline (speedup 1.0000x reference)
"""Trainium2 Bass kernel for GQA attention (B=4, S=2048, D=768, H=12, KVH=4, HD=64).

Sharding: 2 cores per batch. Each core computes all 12 heads for 4 query
chunks of 256 rows (role 0: chunks {0,2,4,6}, role 1: {1,3,5,7}) against the
full K/V of its batch, plus the complete wo projection for its rows. Output
shards are concatenated on the host; no collectives.

All cores run the same graph; causal structure differences between roles are
data-driven (host-built multiplicative masks applied to exp(scores); only 4
distinct [128, 256] masks exist per role since the diagonal offset
256*role - 128*rr is independent of the query chunk).

v3 performance structure:
- rope outputs land at partition base 0 without SBUF->SBUF split DMAs: the
  even head of each pair is an aligned scalar-engine copy of ro[0:64]; the
  odd head goes through a 128x128 half-exchange matmul first. (Matmul
  OPERANDS at partition base 64 hang this runtime when sustained — verified
  by micro-test — so all weights/rhs stay at base 0.)
- softmax denominators: Vector/gpsimd bf16 accumulation of exp tiles + four
  256-col ones-matmuls per (s, grp); no per-kt PE denominator streams.
- input DMAs are chunked and issued round-robin across sync(4 queues),
  scalar(4), gpsimd(8) in deadline order — each queue serializes at ~21GB/s.
- the PE drops to a 1.2GHz p-state if it ever idles; exp on the scalar engine
  (1017ns per [128,1024] tile) is the attention cadence, so each kt iteration
  is padded with deferred wo mi-chunks and, where none are available, idle
  identity matmuls to keep PE work/kt above the exp cadence.
- wo runs as per-s 256-col mi-chunks spread through the following s's
  attention; only the last chunk (s=3) is a tail, double-buffered across two
  psum pools.
"""

import sys

if "/opt/trn_rl_repo" not in sys.path:
    sys.path.insert(0, "/opt/trn_rl_repo")

import os

import numpy as np
import ml_dtypes

import concourse.bass as bass
import concourse.tile as tile
from concourse import bacc, mybir

F32 = mybir.dt.float32
BF16 = mybir.dt.bfloat16

B, S, D = 4, 2048, 768
H, KVH, HD = 12, 4, 64

PAIRS = [(0, 1), (2, 3), (4, 5), (6, 7), (8, 9), (10, 11)]
GROUPS = [(0, 1), (2, 3), (4, 5)]  # pair indices per group


def _kv(h):
    return h // 3


def build_nc(phases=3):
    use_fill = os.environ.get("K_FILLERS", "1") == "1"
    use_gps = os.environ.get("K_GPSADD", "1") == "1"
    use_rep4 = os.environ.get("K_REP4", "1") == "1"
    nc = bacc.Bacc(None, target_bir_lowering=False)

    xT = nc.dram_tensor("xT", [D, S], BF16, kind="ExternalInput")
    xqT = nc.dram_tensor("xqT", [D, 1024], BF16, kind="ExternalInput")
    wq = nc.dram_tensor("wq", [D, H * HD], BF16, kind="ExternalInput")
    wk = nc.dram_tensor("wk", [D, KVH * HD], BF16, kind="ExternalInput")
    wv = nc.dram_tensor("wv", [D, KVH * HD], BF16, kind="ExternalInput")
    wo = nc.dram_tensor("wo", [H * HD, D], BF16, kind="ExternalInput")
    kcs = nc.dram_tensor("kcs", [64, S], BF16, kind="ExternalInput")  # [cos;sin]
    qcs = nc.dram_tensor("qcs", [64, 1024], BF16, kind="ExternalInput")
    masks = nc.dram_tensor("masks", [4, 128, 256], BF16, kind="ExternalInput")
    out = nc.dram_tensor("out", [D, 1024], BF16, kind="ExternalOutput")

    # swap matrix: exchanges 32-partition blocks 0<->1, 2<->3 (rope pairs)
    SW = np.zeros((128, 128), ml_dtypes.bfloat16)
    for blk in range(4):
        srcb = blk ^ 1
        for i in range(32):
            SW[blk * 32 + i, srcb * 32 + i] = 1.0
    sw_dram = nc.inline_tensor(SW, name="swconst")
    # full-half exchange: out partition p <- in partition (p+64)%128
    XC = np.zeros((128, 128), ml_dtypes.bfloat16)
    for p in range(128):
        XC[(p + 64) % 128, p] = 1.0
    xc_dram = nc.inline_tensor(XC, name="xchconst")
    sign = np.zeros((128, 1), np.float32)
    for blk in range(4):
        sign[blk * 32:(blk + 1) * 32] = -1.0 if blk % 2 == 0 else 1.0
    sign_dram = nc.inline_tensor(sign, name="signconst")
    id_dram = nc.inline_tensor(np.eye(128, dtype=ml_dtypes.bfloat16),
                               name="idconst")

    def mm(out_ap, lhsT, rhs, start, stop, tile_position=None):
        nc.tensor.matmul(
            out_ap, lhsT, rhs,
            start=start, stop=stop,
            tile_position=tile_position,
            skip_group_check=True,
        )

    def bcast4(engine, dst, src_dram, row0, col0, width, nrow=32):
        # replicate [nrow, width] of src 128//nrow times across partitions
        base = src_dram[row0:row0 + nrow, col0:col0 + width]
        ap = bass.AP(
            tensor=base.tensor,
            offset=base.offset,
            ap=[[0, 128 // nrow]] + list(base.ap),
        )
        engine.dma_start(out=dst, in_=ap)

    def rep4(base):
        # view a [128, 256] tile as [128, 1024] by repeating the free dim 4x
        return bass.AP(
            tensor=base.tensor,
            offset=base.offset,
            ap=[list(base.ap)[0], [0, 4], list(base.ap)[1]],
        )

    with tile.TileContext(nc) as tc:
        with tc.tile_pool(name="persist", bufs=1) as persist:
            qT = persist.tile([64, H, 1024], BF16)
            kT = persist.tile([64, KVH, S], BF16)
            V = persist.tile([128, 16, 256], BF16)
            wk_sb = persist.tile([128, 6, 256], BF16)
            wv_sb = persist.tile([128, 6, 256], BF16)
            sw_sb = persist.tile([128, 128], BF16)
            xc_sb = persist.tile([128, 128], BF16)
            id_sb = persist.tile([128, 128], BF16)
            sign_sb = persist.tile([128, 1], F32)
            ones64 = persist.tile([128, 64], BF16)
            cosk = persist.tile([128, S], BF16)
            sink = persist.tile([128, S], BF16)
            cosq = persist.tile([128, 1024], BF16)
            sinq = persist.tile([128, 1024], BF16)
            masks_sb = persist.tile([128, 4, 256 if use_rep4 else 1024],
                                    BF16)
            wo_sb = persist.tile([128, 6, D], BF16)
            attnT = persist.tile([128, 6, 1024], BF16)

            # ---- input DMA issue: EDF round-robin over engine queues ----
            # sync has 4 hw queues, scalar 4, gpsimd 8 (software DGE); each
            # queue serializes its transfers, so spread by deadline.
            nc.sync.dma_start(out=id_sb[:, :], in_=id_dram[:, :])
            nc.sync.dma_start(out=sw_sb[:, :], in_=sw_dram[:, :])
            nc.sync.dma_start(out=xc_sb[:, :], in_=xc_dram[:, :])
            nc.sync.dma_start(out=sign_sb[:, :], in_=sign_dram[:, :])
            # gpsimd round 1: k rope tables (needed from ~6us, chunk order)
            for c in range(4):
                bcast4(nc.gpsimd, cosk[:, c * 512:(c + 1) * 512], kcs, 0,
                       c * 512, 512)
                bcast4(nc.gpsimd, sink[:, c * 512:(c + 1) * 512], kcs, 32,
                       c * 512, 512)
            # scalar round 1: wk bundled + first xT tail chunks + masks
            src = wk[:, :]
            nc.scalar.dma_start(
                out=wk_sb[:, :, :],
                in_=bass.AP(tensor=src.tensor, offset=src.offset,
                            ap=[[256, 128], [256 * 128, 6], [1, 256]]))
            src = wv[:, :]
            nc.scalar.dma_start(
                out=wv_sb[:, :, :],
                in_=bass.AP(tensor=src.tensor, offset=src.offset,
                            ap=[[256, 128], [256 * 128, 6], [1, 256]]))
            nc.vector.memset(ones64[:, :], 1.0)

            def load_masks():
                if use_rep4:
                    src = masks[:, :, :]
                    nc.scalar.dma_start(
                        out=masks_sb[:, :, :],
                        in_=bass.AP(tensor=src.tensor, offset=src.offset,
                                    ap=[[256, 128], [128 * 256, 4],
                                        [1, 256]]))
                else:
                    for i in range(4):
                        base = masks[i, :, :]
                        rep = bass.AP(tensor=base.tensor, offset=base.offset,
                                      ap=[list(base.ap)[0], [0, 4],
                                          list(base.ap)[1]])
                        nc.scalar.dma_start(out=masks_sb[:, i, :], in_=rep)

            # ---------------- Phase 1: projections + rope ----------------
            with tc.tile_pool(name="p1", bufs=1) as p1, \
                 tc.tile_pool(name="tmp", bufs=6) as tmpp, \
                 tc.tile_pool(name="psA", bufs=4, space="PSUM") as psA, \
                 tc.tile_pool(name="psB", bufs=2, space="PSUM") as psB:

                xT_sb = p1.tile([128, 6, S], BF16)
                xqT_sb = p1.tile([128, 6, 1024], BF16)
                wq_sb = p1.tile([128, 6, H * HD], BF16)

                # sync: xT column chunks in consumption order (4 queues)
                for c in range(4):
                    for dt in range(4):
                        nc.sync.dma_start(
                            out=xT_sb[:, dt, c * 512:(c + 1) * 512],
                            in_=xT[dt * 128:(dt + 1) * 128,
                                   c * 512:(c + 1) * 512])
                # scalar: xT dt 4-5 + wq + xqT, interleaved by deadline
                def x_tail(c):
                    for dt in (4, 5):
                        nc.scalar.dma_start(
                            out=xT_sb[:, dt, c * 512:(c + 1) * 512],
                            in_=xT[dt * 128:(dt + 1) * 128,
                                   c * 512:(c + 1) * 512])

                def xq_quarter(dthalf, chalf):
                    # [128, 3 dt, 512] bundle of xqT
                    d0 = 3 * dthalf
                    base = xqT[d0 * 128:(d0 + 3) * 128,
                               chalf * 512:(chalf + 1) * 512]
                    nc.scalar.dma_start(
                        out=xqT_sb[:, d0:d0 + 3,
                                   chalf * 512:(chalf + 1) * 512],
                        in_=bass.AP(tensor=base.tensor, offset=base.offset,
                                    ap=[[1024, 128], [1024 * 128, 3],
                                        [1, 512]]))

                x_tail(0)
                x_tail(1)
                xq_quarter(0, 0)
                xq_quarter(1, 0)
                x_tail(2)
                xq_quarter(0, 1)
                xq_quarter(1, 1)
                x_tail(3)
                load_masks()
                # gpsimd round 2: q rope tables, wq, wo (8 software queues)
                bcast4(nc.gpsimd, cosq[:, :], qcs, 0, 0, 1024)
                bcast4(nc.gpsimd, sinq[:, :], qcs, 32, 0, 1024)
                for dt in range(6):
                    nc.gpsimd.dma_start(out=wq_sb[:, dt, :],
                                        in_=wq[dt * 128:(dt + 1) * 128, :])
                for hd in range(6):
                    nc.gpsimd.dma_start(out=wo_sb[:, hd, :],
                                        in_=wo[hd * 128:(hd + 1) * 128, :])

                # PE warmup: ramps the p-state while DMAs land, and preloads
                # the Exp activation table.
                warm_ps = psA.tile([128, 512], F32, tag="pj")
                for wi in range(36):
                    mm(warm_ps[:, 0:128], id_sb[:, :], id_sb[:, :],
                       start=(wi == 0), stop=(wi == 35))
                warm_sb = tmpp.tile([128, 512], F32, tag="ta")
                nc.scalar.activation(
                    out=warm_sb[:, 0:128], in_=warm_ps[:, 0:128],
                    func=mybir.ActivationFunctionType.Exp, scale=0.01)

                def rope_pair(proj_ps, cos_t, sin_t):
                    # rope on a [128, 512] pair tile; returns (sw_ps, ta):
                    # result = sign * sw_ps + ta, to be combined per-half
                    ta = tmpp.tile([128, 512], F32, tag="ta")
                    tb = tmpp.tile([128, 512], BF16, tag="tb")
                    nc.vector.tensor_mul(ta[:, :], proj_ps, cos_t)
                    nc.vector.tensor_mul(tb[:, :], proj_ps, sin_t)
                    sw_ps = psB.tile([128, 512], F32, tag="b")
                    mm(sw_ps[:, :], sw_sb[:, :], tb[:, :], start=True,
                       stop=True)
                    return sw_ps, ta

                def stt_half(dst, sw_ps, ta, p0):
                    nc.vector.scalar_tensor_tensor(
                        out=dst,
                        in0=sw_ps[p0:p0 + 64, :],
                        scalar=sign_sb[p0:p0 + 64, 0:1],
                        in1=ta[p0:p0 + 64, :],
                        op0=mybir.AluOpType.mult,
                        op1=mybir.AluOpType.add,
                    )

                def rope_store(sw_ps, ta, cs, dst_even, dst_odd):
                    # combine rope halves into ro, then land both heads at
                    # partition base 0: even half is an aligned copy; odd half
                    # goes through the half-exchange matmul first (base-64
                    # matmul OPERANDS hang this runtime, so weights/rhs must
                    # stay at partition 0 — the exchange runs on data instead)
                    ro = tmpp.tile([128, 512], BF16, tag="ro")
                    stt_half(ro[0:64, :], sw_ps, ta, 0)
                    stt_half(ro[64:128, :], sw_ps, ta, 64)
                    nc.scalar.copy(dst_even, ro[0:64, :])
                    fs_ps = psB.tile([128, 512], F32, tag="b")
                    mm(fs_ps[:, :], xc_sb[:, :], ro[:, :], start=True,
                       stop=True)
                    nc.scalar.copy(dst_odd, fs_ps[0:64, :])

                def k_chunk(c):
                    cs = slice(c * 512, (c + 1) * 512)
                    for m in range(2):
                        k_ps = psA.tile([128, 512], F32, tag="pj")
                        for dt in range(6):
                            mm(k_ps[:, :], wk_sb[:, dt, m * 128:(m + 1) * 128],
                               xT_sb[:, dt, cs],
                               start=(dt == 0), stop=(dt == 5))
                        sw_ps, ta = rope_pair(k_ps[:, :], cosk[:, cs],
                                              sink[:, cs])
                        rope_store(sw_ps, ta, cs,
                                   kT[:, 2 * m, cs], kT[:, 2 * m + 1, cs])

                def q_chunk(c):
                    cs = slice(c * 512, (c + 1) * 512)
                    for t in range(6):
                        q_ps = psA.tile([128, 512], F32, tag="pj")
                        for dt in range(6):
                            mm(q_ps[:, :], wq_sb[:, dt, t * 128:(t + 1) * 128],
                               xqT_sb[:, dt, cs],
                               start=(dt == 0), stop=(dt == 5))
                        sw_ps, ta = rope_pair(q_ps[:, :], cosq[:, cs],
                                              sinq[:, cs])
                        rope_store(sw_ps, ta, cs,
                                   qT[:, 2 * t, cs], qT[:, 2 * t + 1, cs])

                def v_tiles(c):
                    for st in range(4 * c, 4 * c + 4):
                        v_ps = psA.tile([128, 256], F32, tag="pj")
                        for dt in range(6):
                            mm(v_ps[:, :],
                               xT_sb[:, dt, st * 128:(st + 1) * 128],
                               wv_sb[:, dt, :], start=(dt == 0), stop=(dt == 5))
                        nc.scalar.copy(V[:, st, :], v_ps[:, :])

                k_chunk(0)
                v_tiles(0)
                k_chunk(1)
                v_tiles(1)
                q_chunk(0)
                k_chunk(2)
                v_tiles(2)
                q_chunk(1)
                k_chunk(3)
                v_tiles(3)

            if phases == 1:
                for t in range(6):
                    nc.sync.dma_start(
                        out=out[t * 128:t * 128 + 64, :],
                        in_=qT[:, 2 * t, :])
                    nc.sync.dma_start(
                        out=out[t * 128 + 64:(t + 1) * 128, :],
                        in_=qT[:, 2 * t + 1, :])

            # ---------------- Phase 2: attention + wo ----------------
            if phases >= 2:
              with tc.tile_pool(name="expp", bufs=3) as expp, \
                   tc.tile_pool(name="dacc", bufs=2) as dacc, \
                   tc.tile_pool(name="nrm", bufs=4) as nrm, \
                   tc.tile_pool(name="zsb", bufs=3) as zsb, \
                   tc.tile_pool(name="psSC", bufs=2, space="PSUM") as psSC, \
                   tc.tile_pool(name="psPV", bufs=2, space="PSUM") as psPV, \
                   tc.tile_pool(name="psDN", bufs=1, space="PSUM") as psDN, \
                   tc.tile_pool(name="psZ", bufs=1, space="PSUM") as psZ:

                def wo_mi(sq, mi, pool):
                    # one 128-row wo chunk for query chunk sq (256 cols)
                    z_ps = pool.tile([128, 256], F32, tag="dn" if pool is psDN else "z")
                    for hd in range(6):
                        mm(z_ps[:, :],
                           wo_sb[:, hd, mi * 128:(mi + 1) * 128],
                           attnT[:, hd, sq * 256:(sq + 1) * 256],
                           start=(hd == 0), stop=(hd == 5))
                    z_sb = zsb.tile([128, 256], BF16, tag="z")
                    nc.vector.tensor_copy(z_sb[:, :], z_ps[:, :])
                    nc.sync.dma_start(
                        out=out[mi * 128:(mi + 1) * 128,
                                sq * 256:(sq + 1) * 256],
                        in_=z_sb[:, :])

                def filler(cols):
                    # keeps the PE p-state ramped when real work/kt is below
                    # the scalar-engine exp cadence
                    f_ps = psDN.tile([128, 512], F32, tag="dn")
                    mm(f_ps[:, 0:cols], id_sb[:, :], cosk[:, 0:cols],
                       start=True, stop=True)

                def emit_pv(grp, pv_ps, tgt, kt, n_kt):
                    for pi, p in enumerate(grp):
                        for half in range(2):
                            h = PAIRS[p][half]
                            kv = _kv(h)
                            obase = (h % 2) * 64
                            o = pi * 512 + half * 256
                            mm(pv_ps[obase:obase + 64, pi, :],
                               V[:, kt, kv * 64:(kv + 1) * 64],
                               tgt[:, o:o + 256],
                               start=(kt == 0 and pi == 0),
                               stop=(kt == n_kt - 1),
                               tile_position=(0, obase))

                def make_finalize(s, grp, acc, pv_ps, last_tgt, n_kt):
                    def fin():
                        # flush the software-pipelined last PV, then reduce
                        emit_pv(grp, pv_ps, last_tgt, n_kt - 1, n_kt)
                        den_ps = psDN.tile([128, 2, 256], F32, tag="dn")
                        for pi, p in enumerate(grp):
                            for half in range(2):
                                obase = half * 64
                                o = pi * 512 + half * 256
                                mm(den_ps[obase:obase + 64, pi, :],
                                   ones64[:, :], acc[:, o:o + 256],
                                   start=(pi == 0), stop=True,
                                   tile_position=(0, obase))
                        rec = nrm.tile([128, 2, 256], F32, tag="rec")
                        nc.vector.reciprocal_approx_fast(
                            rec[:, :, :], den_ps[:, :, :])
                        for pi, p in enumerate(grp):
                            nc.vector.tensor_mul(
                                attnT[:, p, s * 256:(s + 1) * 256],
                                pv_ps[:, pi, :],
                                rec[:, pi, :])
                    return fin

                pending = None
                for s in range(4):
                    n_kt = 4 * s + 4
                    # schedule of deferred PE work for this s: wo chunks of
                    # s-1 spread evenly over this s's kt slots, fillers on a
                    # subset of the rest (keep avg cols/kt >= ~2450)
                    n_slot = 3 * n_kt
                    wo_slots = {}
                    if phases >= 3 and s >= 1:
                        for mi in range(6):
                            wo_slots[((2 * mi + 1) * n_slot) // 12] = mi
                    fill_slots = set()
                    if s == 0:
                        fill_slots = set(range(n_slot))
                        fill_cols = 512
                    elif s == 2:
                        fill_slots = {j for j in range(n_slot)
                                      if j % 3 == 1 and j not in wo_slots}
                        fill_cols = 256
                    elif s == 3:
                        fill_slots = {j for j in range(n_slot)
                                      if j % 2 == 1 and j not in wo_slots}
                        fill_cols = 256

                    for gi, grp in enumerate(GROUPS):
                        acc = dacc.tile([128, 1024], BF16, tag="acc")
                        pv_ps = psPV.tile([128, 2, 256], F32)
                        prev_tgt = None
                        for kt in range(n_kt):
                            slot = gi * n_kt + kt
                            sc_ps = psSC.tile([128, 1024], F32)
                            # scores (transposed): [k 128, q 256] per head
                            for pi, p in enumerate(grp):
                                hA, hB = PAIRS[p]
                                o = pi * 512
                                if _kv(hA) == _kv(hB):
                                    mm(sc_ps[:, o:o + 512],
                                       kT[:, _kv(hA),
                                          kt * 128:(kt + 1) * 128],
                                       qT[:, hA:hA + 2,
                                          s * 256:(s + 1) * 256],
                                       start=True, stop=True)
                                else:
                                    for half, h in enumerate((hA, hB)):
                                        mm(sc_ps[:, o + half * 256:
                                                 o + half * 256 + 256],
                                           kT[:, _kv(h),
                                              kt * 128:(kt + 1) * 128],
                                           qT[:, h, s * 256:(s + 1) * 256],
                                           start=(half == 0), stop=True)
                            tgt = expp.tile([128, 1024], BF16, tag="e")
                            nc.scalar.activation(
                                out=tgt[:, :],
                                in_=sc_ps[:, :],
                                func=mybir.ActivationFunctionType.Exp,
                                scale=0.125,
                            )
                            if kt >= 4 * s:
                                mk = masks_sb[:, kt - 4 * s, :]
                                nc.vector.tensor_mul(
                                    tgt[:, :], tgt[:, :],
                                    rep4(mk) if use_rep4 else mk)
                            # finalize previous grp between this grp's first
                            # scores and its first PV matmuls
                            if kt == 0 and pending is not None:
                                pending()
                                pending = None
                            if kt == 1:
                                # first accumulation combines kt0+kt1 (kt0's
                                # exp can't live in acc: the pipelined PV of
                                # kt0 still reads it at kt1)
                                nc.vector.tensor_add(
                                    acc[:, :], prev_tgt[:, :], tgt[:, :])
                            elif kt > 1:
                                # masked-tail adds of the long chunks go to
                                # the otherwise-idle gpsimd engine
                                if use_gps and s >= 2 and kt >= 4 * s:
                                    nc.gpsimd.tensor_add(
                                        acc[:, :], acc[:, :], tgt[:, :])
                                else:
                                    nc.vector.tensor_add(
                                        acc[:, :], acc[:, :], tgt[:, :])
                            # software pipelining: PV of the PREVIOUS kt, so
                            # the in-order PE never blocks on this kt's
                            # exp->mask chain
                            if prev_tgt is not None:
                                emit_pv(grp, pv_ps, prev_tgt, kt - 1, n_kt)
                            prev_tgt = tgt
                            if slot in wo_slots:
                                wo_mi(s - 1, wo_slots[slot], psZ)
                            elif use_fill and slot in fill_slots:
                                filler(fill_cols)
                        pending = make_finalize(s, grp, acc, pv_ps, prev_tgt,
                                                n_kt)

                if pending is not None:
                    pending()
                    pending = None
                if phases >= 3:
                    # tail: wo of s=3, alternating psum pools so the PE never
                    # waits on the drain copies
                    for mi in range(6):
                        wo_mi(3, mi, psZ if mi % 2 == 0 else psDN)

                if phases == 2:
                    for t in range(6):
                        nc.sync.dma_start(
                            out=out[t * 128:(t + 1) * 128, :],
                            in_=attnT[:, t, :])

    nc.compile()
    return nc


# ---------------------------------------------------------------------------
# host side
# ---------------------------------------------------------------------------

def _permute_cols(w, nheads):
    """Deinterleave rope pairs within each head: even dims then odd dims."""
    cols = []
    for h in range(nheads):
        blk = w[:, h * HD:(h + 1) * HD]
        cols.append(blk[:, 0::2])
        cols.append(blk[:, 1::2])
    return np.ascontiguousarray(np.concatenate(cols, axis=1))


def make_in_maps(x, wq, wk, wv, wo, freqs_cos, freqs_sin):
    bf = ml_dtypes.bfloat16
    wq_p = _permute_cols(np.asarray(wq, np.float32), H).astype(bf)
    wk_p = _permute_cols(np.asarray(wk, np.float32), KVH).astype(bf)
    wv_b = np.ascontiguousarray(np.asarray(wv, np.float32)).astype(bf)
    wo_b = np.ascontiguousarray(np.asarray(wo, np.float32)).astype(bf)

    cosT = np.ascontiguousarray(np.asarray(freqs_cos, np.float32).T)  # [32, S]
    sinT = np.ascontiguousarray(np.asarray(freqs_sin, np.float32).T)
    kcs = np.ascontiguousarray(np.concatenate([cosT, sinT], axis=0))  # [64, S]

    in_maps = []
    for core in range(8):
        b, role = core // 2, core % 2
        xT = np.ascontiguousarray(np.asarray(x[b], np.float32).T)
        q_rows = np.concatenate(
            [np.arange(256 * (2 * s + role), 256 * (2 * s + role) + 256)
             for s in range(4)])
        xqT = np.ascontiguousarray(xT[:, q_rows])
        qcs = np.ascontiguousarray(kcs[:, q_rows])
        # only 4 distinct masks: diagonal offset 256*role - 128*rr
        m = np.zeros((4, 128, 256), np.float32)
        ar = np.arange(256)[None, :]
        pr = np.arange(128)[:, None]
        for rr in range(4):
            m[rr] = (pr <= ar + 256 * role - 128 * rr).astype(np.float32)
        in_maps.append({
            "xT": xT.astype(bf),
            "xqT": xqT.astype(bf),
            "wq": wq_p,
            "wk": wk_p,
            "wv": wv_b,
            "wo": wo_b,
            "kcs": kcs.astype(bf),
            "qcs": qcs.astype(bf),
            "masks": m.astype(bf),
        })
    return in_maps


_NC_CACHE = {}


def kernel(x, wq, wk, wv, wo, freqs_cos, freqs_sin, mask_attention,
           start_pos=0, inference=0, **_ignored):
    from concourse.bass_utils import run_bass_kernel_spmd

    in_maps = make_in_maps(np.asarray(x, np.float32), wq, wk, wv, wo,
                           freqs_cos, freqs_sin)
    if "nc" not in _NC_CACHE:
        _NC_CACHE["nc"] = build_nc()
    nc = _NC_CACHE["nc"]
    res = run_bass_kernel_spmd(nc, in_maps, core_ids=list(range(8)))
    outs = res.results
    out_full = np.zeros((B, S, D), np.float32)
    for core in range(8):
        b, role = core // 2, core % 2
        zT = np.asarray(outs[core]["out"], np.float32)  # [768, 1024]
        for s in range(4):
            j = 2 * s + role
            out_full[b, 256 * j:256 * j + 256, :] = zT[:, 256 * s:256 * s + 256].T
    return out_full


# revision 30
# speedup vs baseline: 1.2428x; 1.2428x over previous
"""Trainium2 Bass kernel for GQA attention (B=4, S=2048, D=768, H=12, KVH=4, HD=64).

Sharding: 2 cores per batch. Each core computes all 12 heads for 4 query
chunks of 256 rows (role 0: chunks {0,2,4,6}, role 1: {1,3,5,7}) against the
full K/V of its batch, plus the complete wo projection for its rows. Output
shards are concatenated on the host; no collectives.

All cores run the same graph; causal structure differences between roles are
data-driven (host-built multiplicative masks applied to exp(scores); only 4
distinct [128, 256] masks exist per role since the diagonal offset
256*role - 128*rr is independent of the query chunk).

v3 performance structure:
- rope outputs land at partition base 0 without SBUF->SBUF split DMAs: the
  even head of each pair is an aligned scalar-engine copy of ro[0:64]; the
  odd head goes through a 128x128 half-exchange matmul first. (Matmul
  OPERANDS at partition base 64 hang this runtime when sustained — verified
  by micro-test — so all weights/rhs stay at base 0.)
- softmax denominators: Vector/gpsimd bf16 accumulation of exp tiles + four
  256-col ones-matmuls per (s, grp); no per-kt PE denominator streams.
- input DMAs are chunked and issued round-robin across sync(4 queues),
  scalar(4), gpsimd(8) in deadline order — each queue serializes at ~21GB/s.
- the PE drops to a 1.2GHz p-state if it ever idles; exp on the scalar engine
  (1017ns per [128,1024] tile) is the attention cadence, so each kt iteration
  is padded with deferred wo mi-chunks and, where none are available, idle
  identity matmuls to keep PE work/kt above the exp cadence.
- wo runs as per-s 256-col mi-chunks spread through the following s's
  attention; only the last chunk (s=3) is a tail, double-buffered across two
  psum pools.
"""

import sys

if "/opt/trn_rl_repo" not in sys.path:
    sys.path.insert(0, "/opt/trn_rl_repo")

import os

import numpy as np
import ml_dtypes

import concourse.bass as bass
import concourse.tile as tile
from concourse import bacc, mybir

F32 = mybir.dt.float32
BF16 = mybir.dt.bfloat16

B, S, D = 4, 2048, 768
H, KVH, HD = 12, 4, 64

PAIRS = [(0, 1), (2, 3), (4, 5), (6, 7), (8, 9), (10, 11)]
GROUPS = [(0, 1), (2, 3), (4, 5)]  # pair indices per group


def _kv(h):
    return h // 3


def build_nc(phases=3):
    use_fill = os.environ.get("K_FILLERS", "1") == "1"
    use_gps = os.environ.get("K_GPSADD", "1") == "1"
    use_rep4 = os.environ.get("K_REP4", "1") == "1"
    nc = bacc.Bacc(None, target_bir_lowering=False)

    xT = nc.dram_tensor("xT", [D, S], BF16, kind="ExternalInput")
    xqT = nc.dram_tensor("xqT", [D, 1024], BF16, kind="ExternalInput")
    wq = nc.dram_tensor("wq", [D, H * HD], BF16, kind="ExternalInput")
    wk = nc.dram_tensor("wk", [D, KVH * HD], BF16, kind="ExternalInput")
    wv = nc.dram_tensor("wv", [D, KVH * HD], BF16, kind="ExternalInput")
    wo = nc.dram_tensor("wo", [H * HD, D], BF16, kind="ExternalInput")
    kcs = nc.dram_tensor("kcs", [64, S], BF16, kind="ExternalInput")  # [cos;sin]
    qcs = nc.dram_tensor("qcs", [64, 1024], BF16, kind="ExternalInput")
    masks = nc.dram_tensor("masks", [4, 128, 256], BF16, kind="ExternalInput")
    out = nc.dram_tensor("out", [D, 1024], BF16, kind="ExternalOutput")

    # swap matrix: exchanges 32-partition blocks 0<->1, 2<->3 (rope pairs)
    SW = np.zeros((128, 128), ml_dtypes.bfloat16)
    for blk in range(4):
        srcb = blk ^ 1
        for i in range(32):
            SW[blk * 32 + i, srcb * 32 + i] = 1.0
    sw_dram = nc.inline_tensor(SW, name="swconst")
    # full-half exchange: out partition p <- in partition (p+64)%128
    XC = np.zeros((128, 128), ml_dtypes.bfloat16)
    for p in range(128):
        XC[(p + 64) % 128, p] = 1.0
    xc_dram = nc.inline_tensor(XC, name="xchconst")
    sign = np.zeros((128, 1), np.float32)
    for blk in range(4):
        sign[blk * 32:(blk + 1) * 32] = -1.0 if blk % 2 == 0 else 1.0
    sign_dram = nc.inline_tensor(sign, name="signconst")
    id_dram = nc.inline_tensor(np.eye(128, dtype=ml_dtypes.bfloat16),
                               name="idconst")

    def mm(out_ap, lhsT, rhs, start, stop, tile_position=None):
        nc.tensor.matmul(
            out_ap, lhsT, rhs,
            start=start, stop=stop,
            tile_position=tile_position,
            skip_group_check=True,
        )

    def bcast4(engine, dst, src_dram, row0, col0, width, nrow=32):
        # replicate [nrow, width] of src 128//nrow times across partitions
        base = src_dram[row0:row0 + nrow, col0:col0 + width]
        ap = bass.AP(
            tensor=base.tensor,
            offset=base.offset,
            ap=[[0, 128 // nrow]] + list(base.ap),
        )
        engine.dma_start(out=dst, in_=ap)

    def rep4(base):
        # view a [128, 256] tile as [128, 1024] by repeating the free dim 4x
        return bass.AP(
            tensor=base.tensor,
            offset=base.offset,
            ap=[list(base.ap)[0], [0, 4], list(base.ap)[1]],
        )

    with tile.TileContext(nc) as tc:
        with tc.tile_pool(name="persist", bufs=1) as persist:
            qT = persist.tile([64, H, 1024], BF16)
            kT = persist.tile([64, KVH, S], BF16)
            V = persist.tile([128, 16, 256], BF16)
            wk_sb = persist.tile([128, 6, 256], BF16)
            wv_sb = persist.tile([128, 6, 256], BF16)
            sw_sb = persist.tile([128, 128], BF16)
            xc_sb = persist.tile([128, 128], BF16)
            id_sb = persist.tile([128, 128], BF16)
            sign_sb = persist.tile([128, 1], F32)
            ones64 = persist.tile([128, 64], BF16)
            cosk = persist.tile([128, S], BF16)
            sink = persist.tile([128, S], BF16)
            cosq = persist.tile([128, 1024], BF16)
            sinq = persist.tile([128, 1024], BF16)
            masks_sb = persist.tile([128, 4, 256 if use_rep4 else 1024],
                                    BF16)
            wo_sb = persist.tile([128, 6, D], BF16)
            attnT = persist.tile([128, 6, 1024], BF16)

            # ---- input DMA issue: EDF round-robin over engine queues ----
            # sync has 4 hw queues, scalar 4, gpsimd 8 (software DGE); each
            # queue serializes its transfers, so spread by deadline.
            nc.sync.dma_start(out=id_sb[:, :], in_=id_dram[:, :])
            nc.sync.dma_start(out=sw_sb[:, :], in_=sw_dram[:, :])
            nc.sync.dma_start(out=xc_sb[:, :], in_=xc_dram[:, :])
            nc.sync.dma_start(out=sign_sb[:, :], in_=sign_dram[:, :])
            # gpsimd round 1: first k rope table chunks (needed from ~8us)
            for c in range(2):
                bcast4(nc.gpsimd, cosk[:, c * 512:(c + 1) * 512], kcs, 0,
                       c * 512, 512)
                bcast4(nc.gpsimd, sink[:, c * 512:(c + 1) * 512], kcs, 32,
                       c * 512, 512)
            # scalar round 1: wk bundled + first xT tail chunks + masks
            src = wk[:, :]
            nc.scalar.dma_start(
                out=wk_sb[:, :, :],
                in_=bass.AP(tensor=src.tensor, offset=src.offset,
                            ap=[[256, 128], [256 * 128, 6], [1, 256]]))
            src = wv[:, :]
            nc.scalar.dma_start(
                out=wv_sb[:, :, :],
                in_=bass.AP(tensor=src.tensor, offset=src.offset,
                            ap=[[256, 128], [256 * 128, 6], [1, 256]]))
            nc.vector.memset(ones64[:, :], 1.0)

            def load_masks():
                if use_rep4:
                    src = masks[:, :, :]
                    nc.scalar.dma_start(
                        out=masks_sb[:, :, :],
                        in_=bass.AP(tensor=src.tensor, offset=src.offset,
                                    ap=[[256, 128], [128 * 256, 4],
                                        [1, 256]]))
                else:
                    for i in range(4):
                        base = masks[i, :, :]
                        rep = bass.AP(tensor=base.tensor, offset=base.offset,
                                      ap=[list(base.ap)[0], [0, 4],
                                          list(base.ap)[1]])
                        nc.scalar.dma_start(out=masks_sb[:, i, :], in_=rep)

            # ---------------- Phase 1: projections + rope ----------------
            with tc.tile_pool(name="p1", bufs=1) as p1, \
                 tc.tile_pool(name="tmp", bufs=6) as tmpp, \
                 tc.tile_pool(name="psA", bufs=4, space="PSUM") as psA, \
                 tc.tile_pool(name="psB", bufs=2, space="PSUM") as psB:

                xT_sb = p1.tile([128, 6, S], BF16)
                xqT_sb = p1.tile([128, 6, 1024], BF16)
                wq_sb = p1.tile([128, 6, H * HD], BF16)

                def wq_piece(dt, engine):
                    engine.dma_start(out=wq_sb[:, dt, :],
                                     in_=wq[dt * 128:(dt + 1) * 128, :])

                # sync: xT column chunks in consumption order (4 queues),
                # with two wq pieces slotted between c1 and c2
                for c in range(4):
                    if c == 2:
                        wq_piece(0, nc.sync)
                        wq_piece(1, nc.sync)
                    for dt in range(4):
                        nc.sync.dma_start(
                            out=xT_sb[:, dt, c * 512:(c + 1) * 512],
                            in_=xT[dt * 128:(dt + 1) * 128,
                                   c * 512:(c + 1) * 512])
                # scalar: xT dt 4-5 + wq + xqT, interleaved by deadline
                def x_tail(c):
                    for dt in (4, 5):
                        nc.scalar.dma_start(
                            out=xT_sb[:, dt, c * 512:(c + 1) * 512],
                            in_=xT[dt * 128:(dt + 1) * 128,
                                   c * 512:(c + 1) * 512])

                def xq_quarter(dthalf, chalf):
                    # [128, 3 dt, 512] bundle of xqT
                    d0 = 3 * dthalf
                    base = xqT[d0 * 128:(d0 + 3) * 128,
                               chalf * 512:(chalf + 1) * 512]
                    nc.scalar.dma_start(
                        out=xqT_sb[:, d0:d0 + 3,
                                   chalf * 512:(chalf + 1) * 512],
                        in_=bass.AP(tensor=base.tensor, offset=base.offset,
                                    ap=[[1024, 128], [1024 * 128, 3],
                                        [1, 512]]))

                x_tail(0)
                x_tail(1)
                xq_quarter(0, 0)
                xq_quarter(1, 0)
                wq_piece(2, nc.scalar)
                wq_piece(3, nc.scalar)
                x_tail(2)
                xq_quarter(0, 1)
                xq_quarter(1, 1)
                x_tail(3)
                load_masks()
                # gpsimd round 2: q rope tables, wq pieces, remaining k rope
                # tables, wo (8 software queues)
                bcast4(nc.gpsimd, cosq[:, :], qcs, 0, 0, 1024)
                bcast4(nc.gpsimd, sinq[:, :], qcs, 32, 0, 1024)
                wq_piece(4, nc.gpsimd)
                wq_piece(5, nc.gpsimd)
                for c in range(2, 4):
                    bcast4(nc.gpsimd, cosk[:, c * 512:(c + 1) * 512], kcs, 0,
                           c * 512, 512)
                    bcast4(nc.gpsimd, sink[:, c * 512:(c + 1) * 512], kcs, 32,
                           c * 512, 512)
                for hd in range(6):
                    nc.gpsimd.dma_start(out=wo_sb[:, hd, :],
                                        in_=wo[hd * 128:(hd + 1) * 128, :])

                # PE warmup: ramps the p-state while DMAs land, and preloads
                # the Exp activation table.
                warm_ps = psA.tile([128, 512], F32, tag="pj")
                for wi in range(36):
                    mm(warm_ps[:, 0:128], id_sb[:, :], id_sb[:, :],
                       start=(wi == 0), stop=(wi == 35))
                warm_sb = tmpp.tile([128, 512], F32, tag="ta")
                nc.scalar.activation(
                    out=warm_sb[:, 0:128], in_=warm_ps[:, 0:128],
                    func=mybir.ActivationFunctionType.Exp, scale=0.01)

                def rope_pair(proj_ps, cos_t, sin_t):
                    # rope on a [128, 512] pair tile; returns (sw_ps, ta):
                    # result = sign * sw_ps + ta, to be combined per-half
                    ta = tmpp.tile([128, 512], F32, tag="ta")
                    tb = tmpp.tile([128, 512], BF16, tag="tb")
                    nc.vector.tensor_mul(ta[:, :], proj_ps, cos_t)
                    nc.vector.tensor_mul(tb[:, :], proj_ps, sin_t)
                    sw_ps = psB.tile([128, 512], F32, tag="b")
                    mm(sw_ps[:, :], sw_sb[:, :], tb[:, :], start=True,
                       stop=True)
                    return sw_ps, ta

                def stt_half(dst, sw_ps, ta, p0):
                    nc.vector.scalar_tensor_tensor(
                        out=dst,
                        in0=sw_ps[p0:p0 + 64, :],
                        scalar=sign_sb[p0:p0 + 64, 0:1],
                        in1=ta[p0:p0 + 64, :],
                        op0=mybir.AluOpType.mult,
                        op1=mybir.AluOpType.add,
                    )

                def rope_store(sw_ps, ta, cs, dst_even, dst_odd):
                    # combine rope halves into ro, then land both heads at
                    # partition base 0: even half is an aligned copy; odd half
                    # goes through the half-exchange matmul first (base-64
                    # matmul OPERANDS hang this runtime, so weights/rhs must
                    # stay at partition 0 — the exchange runs on data instead)
                    ro = tmpp.tile([128, 512], BF16, tag="ro")
                    stt_half(ro[0:64, :], sw_ps, ta, 0)
                    stt_half(ro[64:128, :], sw_ps, ta, 64)
                    nc.scalar.copy(dst_even, ro[0:64, :])
                    fs_ps = psB.tile([128, 512], F32, tag="b")
                    mm(fs_ps[:, :], xc_sb[:, :], ro[:, :], start=True,
                       stop=True)
                    nc.scalar.copy(dst_odd, fs_ps[0:64, :])

                def k_chunk(c):
                    cs = slice(c * 512, (c + 1) * 512)
                    for m in range(2):
                        k_ps = psA.tile([128, 512], F32, tag="pj")
                        for dt in range(6):
                            mm(k_ps[:, :], wk_sb[:, dt, m * 128:(m + 1) * 128],
                               xT_sb[:, dt, cs],
                               start=(dt == 0), stop=(dt == 5))
                        sw_ps, ta = rope_pair(k_ps[:, :], cosk[:, cs],
                                              sink[:, cs])
                        rope_store(sw_ps, ta, cs,
                                   kT[:, 2 * m, cs], kT[:, 2 * m + 1, cs])

                def q_chunk(c):
                    cs = slice(c * 512, (c + 1) * 512)
                    for t in range(6):
                        q_ps = psA.tile([128, 512], F32, tag="pj")
                        for dt in range(6):
                            mm(q_ps[:, :], wq_sb[:, dt, t * 128:(t + 1) * 128],
                               xqT_sb[:, dt, cs],
                               start=(dt == 0), stop=(dt == 5))
                        sw_ps, ta = rope_pair(q_ps[:, :], cosq[:, cs],
                                              sinq[:, cs])
                        rope_store(sw_ps, ta, cs,
                                   qT[:, 2 * t, cs], qT[:, 2 * t + 1, cs])

                def v_tiles(c):
                    for st in range(4 * c, 4 * c + 4):
                        v_ps = psA.tile([128, 256], F32, tag="pj")
                        for dt in range(6):
                            mm(v_ps[:, :],
                               xT_sb[:, dt, st * 128:(st + 1) * 128],
                               wv_sb[:, dt, :], start=(dt == 0), stop=(dt == 5))
                        nc.scalar.copy(V[:, st, :], v_ps[:, :])

                k_chunk(0)
                v_tiles(0)
                k_chunk(1)
                v_tiles(1)
                k_chunk(2)
                v_tiles(2)
                q_chunk(0)
                k_chunk(3)
                v_tiles(3)
                q_chunk(1)

            if phases == 1:
                for t in range(6):
                    nc.sync.dma_start(
                        out=out[t * 128:t * 128 + 64, :],
                        in_=qT[:, 2 * t, :])
                    nc.sync.dma_start(
                        out=out[t * 128 + 64:(t + 1) * 128, :],
                        in_=qT[:, 2 * t + 1, :])

            # ---------------- Phase 2: attention + wo ----------------
            if phases >= 2:
              with tc.tile_pool(name="expp", bufs=3) as expp, \
                   tc.tile_pool(name="dacc", bufs=2) as dacc, \
                   tc.tile_pool(name="nrm", bufs=4) as nrm, \
                   tc.tile_pool(name="zsb", bufs=3) as zsb, \
                   tc.tile_pool(name="psSC", bufs=2, space="PSUM") as psSC, \
                   tc.tile_pool(name="psPV", bufs=2, space="PSUM") as psPV, \
                   tc.tile_pool(name="psDN", bufs=1, space="PSUM") as psDN, \
                   tc.tile_pool(name="psZ", bufs=1, space="PSUM") as psZ:

                def wo_mi(sq, mi, pool):
                    # one 128-row wo chunk for query chunk sq (256 cols)
                    z_ps = pool.tile([128, 256], F32, tag="dn" if pool is psDN else "z")
                    for hd in range(6):
                        mm(z_ps[:, :],
                           wo_sb[:, hd, mi * 128:(mi + 1) * 128],
                           attnT[:, hd, sq * 256:(sq + 1) * 256],
                           start=(hd == 0), stop=(hd == 5))
                    z_sb = zsb.tile([128, 256], BF16, tag="z")
                    nc.scalar.copy(z_sb[:, :], z_ps[:, :])
                    nc.sync.dma_start(
                        out=out[mi * 128:(mi + 1) * 128,
                                sq * 256:(sq + 1) * 256],
                        in_=z_sb[:, :])

                def emit_pv(grp, pv_ps, tgt, kt, n_kt):
                    for pi, p in enumerate(grp):
                        for half in range(2):
                            h = PAIRS[p][half]
                            kv = _kv(h)
                            obase = (h % 2) * 64
                            o = pi * 512 + half * 256
                            mm(pv_ps[obase:obase + 64, pi, :],
                               V[:, kt, kv * 64:(kv + 1) * 64],
                               tgt[:, o:o + 256],
                               start=(kt == 0 and pi == 0),
                               stop=(kt == n_kt - 1),
                               tile_position=(0, obase))

                def emit_den(grp, den_ps, src, first, stop):
                    # four 256-col ones-matmul streams accumulating per-head
                    # denominators into den_ps (partition-matched layout)
                    for pi, p in enumerate(grp):
                        for half in range(2):
                            obase = half * 64
                            o = pi * 512 + half * 256
                            mm(den_ps[obase:obase + 64, pi, :],
                               ones64[:, :], src[:, o:o + 256],
                               start=(first and pi == 0), stop=stop,
                               tile_position=(0, obase))

                # den-PE kts: enough per-kt PE work to keep the engine
                # wait-free (it locks to a low p-state otherwise); the rest
                # accumulate on the Vector engine into acc
                DEN_PE = {
                    0: {0, 1, 2, 3},
                    1: {4, 5, 6, 7},
                    2: {2, 8, 9, 10, 11},
                    3: {2, 3, 4, 5, 12, 13, 14, 15},
                }

                def make_finalize(s, grp, acc, pv_ps, den_ps, last_tgt, n_kt):
                    has_fold = len(DEN_PE[s]) < n_kt
                    def fin():
                        # flush the software-pipelined last PV + den
                        emit_pv(grp, pv_ps, last_tgt, n_kt - 1, n_kt)
                        emit_den(grp, den_ps, last_tgt,
                                 first=(DEN_PE[s] == {n_kt - 1}),
                                 stop=not has_fold)
                        if has_fold:
                            emit_den(grp, den_ps, acc, first=False, stop=True)
                        rec = nrm.tile([128, 2, 256], F32, tag="rec")
                        nc.vector.reciprocal_approx_fast(
                            rec[:, :, :], den_ps[:, :, :])
                        for pi, p in enumerate(grp):
                            nc.vector.tensor_mul(
                                attnT[:, p, s * 256:(s + 1) * 256],
                                pv_ps[:, pi, :],
                                rec[:, pi, :])
                    return fin

                pending = None
                for s in range(4):
                    n_kt = 4 * s + 4
                    # wo chunks of s-1 spread evenly over this s's kt slots
                    n_slot = 3 * n_kt
                    wo_slots = {}
                    if phases >= 3 and s >= 1:
                        for mi in range(6):
                            wo_slots[((2 * mi + 1) * n_slot) // 12] = mi
                    den_pe = DEN_PE[s]

                    for gi, grp in enumerate(GROUPS):
                        acc = dacc.tile([128, 1024], BF16, tag="acc")
                        pv_ps = psPV.tile([128, 2, 256], F32)
                        den_ps = psDN.tile([128, 2, 256], F32, tag="dn")
                        den_first = True
                        prev_tgt = None
                        for kt in range(n_kt):
                            slot = gi * n_kt + kt
                            sc_ps = psSC.tile([128, 1024], F32)
                            # scores (transposed): [k 128, q 256] per head
                            for pi, p in enumerate(grp):
                                hA, hB = PAIRS[p]
                                o = pi * 512
                                if _kv(hA) == _kv(hB):
                                    mm(sc_ps[:, o:o + 512],
                                       kT[:, _kv(hA),
                                          kt * 128:(kt + 1) * 128],
                                       qT[:, hA:hA + 2,
                                          s * 256:(s + 1) * 256],
                                       start=True, stop=True)
                                else:
                                    for half, h in enumerate((hA, hB)):
                                        mm(sc_ps[:, o + half * 256:
                                                 o + half * 256 + 256],
                                           kT[:, _kv(h),
                                              kt * 128:(kt + 1) * 128],
                                           qT[:, h, s * 256:(s + 1) * 256],
                                           start=(half == 0), stop=True)
                            tgt = expp.tile([128, 1024], BF16, tag="e")
                            nc.scalar.activation(
                                out=tgt[:, :],
                                in_=sc_ps[:, :],
                                func=mybir.ActivationFunctionType.Exp,
                                scale=0.125,
                            )
                            if kt >= 4 * s:
                                mk = masks_sb[:, kt - 4 * s, :]
                                nc.vector.tensor_mul(
                                    tgt[:, :], tgt[:, :],
                                    rep4(mk) if use_rep4 else mk)
                            # finalize previous grp between this grp's first
                            # scores and its first PV matmuls
                            if kt == 0 and pending is not None:
                                pending()
                                pending = None
                            if kt not in den_pe:
                                if kt == 1:
                                    # first accumulation combines kt0+kt1
                                    # (kt0's exp can't live in acc: the
                                    # pipelined PV of kt0 still reads it)
                                    nc.vector.tensor_add(
                                        acc[:, :], prev_tgt[:, :], tgt[:, :])
                                elif kt > 1:
                                    nc.vector.tensor_add(
                                        acc[:, :], acc[:, :], tgt[:, :])
                            # software pipelining: PV (+den) of the PREVIOUS
                            # kt, so the in-order PE never blocks on this
                            # kt's exp->mask chain
                            if prev_tgt is not None:
                                emit_pv(grp, pv_ps, prev_tgt, kt - 1, n_kt)
                                if kt - 1 in den_pe:
                                    emit_den(grp, den_ps, prev_tgt,
                                             first=den_first, stop=False)
                                    den_first = False
                            prev_tgt = tgt
                            if slot in wo_slots:
                                wo_mi(s - 1, wo_slots[slot], psZ)
                        pending = make_finalize(s, grp, acc, pv_ps, den_ps,
                                                prev_tgt, n_kt)

                if pending is not None:
                    pending()
                    pending = None
                if phases >= 3:
                    # tail: wo of s=3, alternating psum pools so the PE never
                    # waits on the drain copies
                    for mi in range(6):
                        wo_mi(3, mi, psZ if mi % 2 == 0 else psDN)

                if phases == 2:
                    for t in range(6):
                        nc.sync.dma_start(
                            out=out[t * 128:(t + 1) * 128, :],
                            in_=attnT[:, t, :])

    nc.compile()
    return nc


# ---------------------------------------------------------------------------
# host side
# ---------------------------------------------------------------------------

def _permute_cols(w, nheads):
    """Deinterleave rope pairs within each head: even dims then odd dims."""
    cols = []
    for h in range(nheads):
        blk = w[:, h * HD:(h + 1) * HD]
        cols.append(blk[:, 0::2])
        cols.append(blk[:, 1::2])
    return np.ascontiguousarray(np.concatenate(cols, axis=1))


def make_in_maps(x, wq, wk, wv, wo, freqs_cos, freqs_sin):
    bf = ml_dtypes.bfloat16
    wq_p = _permute_cols(np.asarray(wq, np.float32), H).astype(bf)
    wk_p = _permute_cols(np.asarray(wk, np.float32), KVH).astype(bf)
    wv_b = np.ascontiguousarray(np.asarray(wv, np.float32)).astype(bf)
    wo_b = np.ascontiguousarray(np.asarray(wo, np.float32)).astype(bf)

    cosT = np.ascontiguousarray(np.asarray(freqs_cos, np.float32).T)  # [32, S]
    sinT = np.ascontiguousarray(np.asarray(freqs_sin, np.float32).T)
    kcs = np.ascontiguousarray(np.concatenate([cosT, sinT], axis=0))  # [64, S]

    in_maps = []
    for core in range(8):
        b, role = core // 2, core % 2
        xT = np.ascontiguousarray(np.asarray(x[b], np.float32).T)
        q_rows = np.concatenate(
            [np.arange(256 * (2 * s + role), 256 * (2 * s + role) + 256)
             for s in range(4)])
        xqT = np.ascontiguousarray(xT[:, q_rows])
        qcs = np.ascontiguousarray(kcs[:, q_rows])
        # only 4 distinct masks: diagonal offset 256*role - 128*rr
        m = np.zeros((4, 128, 256), np.float32)
        ar = np.arange(256)[None, :]
        pr = np.arange(128)[:, None]
        for rr in range(4):
            m[rr] = (pr <= ar + 256 * role - 128 * rr).astype(np.float32)
        in_maps.append({
            "xT": xT.astype(bf),
            "xqT": xqT.astype(bf),
            "wq": wq_p,
            "wk": wk_p,
            "wv": wv_b,
            "wo": wo_b,
            "kcs": kcs.astype(bf),
            "qcs": qcs.astype(bf),
            "masks": m.astype(bf),
        })
    return in_maps


_NC_CACHE = {}


def kernel(x, wq, wk, wv, wo, freqs_cos, freqs_sin, mask_attention,
           start_pos=0, inference=0, **_ignored):
    from concourse.bass_utils import run_bass_kernel_spmd

    in_maps = make_in_maps(np.asarray(x, np.float32), wq, wk, wv, wo,
                           freqs_cos, freqs_sin)
    if "nc" not in _NC_CACHE:
        _NC_CACHE["nc"] = build_nc()
    nc = _NC_CACHE["nc"]
    res = run_bass_kernel_spmd(nc, in_maps, core_ids=list(range(8)))
    outs = res.results
    out_full = np.zeros((B, S, D), np.float32)
    for core in range(8):
        b, role = core // 2, core % 2
        zT = np.asarray(outs[core]["out"], np.float32)  # [768, 1024]
        for s in range(4):
            j = 2 * s + role
            out_full[b, 256 * j:256 * j + 256, :] = zT[:, 256 * s:256 * s + 256].T
    return out_full


# revision 31
# speedup vs baseline: 1.3209x; 1.0628x over previous
"""Trainium2 Bass kernel for GQA attention (B=4, S=2048, D=768, H=12, KVH=4, HD=64).

Sharding: 2 cores per batch. Each core computes all 12 heads for 4 query
chunks of 256 rows (role 0: chunks {0,2,4,6}, role 1: {1,3,5,7}) against the
full K/V of its batch, plus the complete wo projection for its rows. Output
shards are concatenated on the host; no collectives.

All cores run the same graph; causal structure differences between roles are
data-driven (host-built multiplicative masks applied to exp(scores); only 4
distinct [128, 256] masks exist per role since the diagonal offset
256*role - 128*rr is independent of the query chunk).

v3 performance structure:
- rope outputs land at partition base 0 without SBUF->SBUF split DMAs: the
  even head of each pair is an aligned scalar-engine copy of ro[0:64]; the
  odd head goes through a 128x128 half-exchange matmul first. (Matmul
  OPERANDS at partition base 64 hang this runtime when sustained — verified
  by micro-test — so all weights/rhs stay at base 0.)
- softmax denominators: Vector/gpsimd bf16 accumulation of exp tiles + four
  256-col ones-matmuls per (s, grp); no per-kt PE denominator streams.
- input DMAs are chunked and issued round-robin across sync(4 queues),
  scalar(4), gpsimd(8) in deadline order — each queue serializes at ~21GB/s.
- the PE drops to a 1.2GHz p-state if it ever idles; exp on the scalar engine
  (1017ns per [128,1024] tile) is the attention cadence, so each kt iteration
  is padded with deferred wo mi-chunks and, where none are available, idle
  identity matmuls to keep PE work/kt above the exp cadence.
- wo runs as per-s 256-col mi-chunks spread through the following s's
  attention; only the last chunk (s=3) is a tail, double-buffered across two
  psum pools.
"""

import sys

if "/opt/trn_rl_repo" not in sys.path:
    sys.path.insert(0, "/opt/trn_rl_repo")

import os

import numpy as np
import ml_dtypes

import concourse.bass as bass
import concourse.tile as tile
from concourse import bacc, mybir

F32 = mybir.dt.float32
BF16 = mybir.dt.bfloat16

B, S, D = 4, 2048, 768
H, KVH, HD = 12, 4, 64

PAIRS = [(0, 1), (2, 3), (4, 5), (6, 7), (8, 9), (10, 11)]
GROUPS = [(0, 1), (2, 3), (4, 5)]  # pair indices per group


def _kv(h):
    return h // 3


def build_nc(phases=3):
    use_fill = os.environ.get("K_FILLERS", "1") == "1"
    use_gps = os.environ.get("K_GPSADD", "1") == "1"
    use_rep4 = os.environ.get("K_REP4", "1") == "1"
    nc = bacc.Bacc(None, target_bir_lowering=False)

    xT = nc.dram_tensor("xT", [D, S], BF16, kind="ExternalInput")
    xqT = nc.dram_tensor("xqT", [D, 1024], BF16, kind="ExternalInput")
    wq = nc.dram_tensor("wq", [D, H * HD], BF16, kind="ExternalInput")
    wk = nc.dram_tensor("wk", [D, KVH * HD], BF16, kind="ExternalInput")
    wv = nc.dram_tensor("wv", [D, KVH * HD], BF16, kind="ExternalInput")
    wo = nc.dram_tensor("wo", [H * HD, D], BF16, kind="ExternalInput")
    kcs = nc.dram_tensor("kcs", [64, S], BF16, kind="ExternalInput")  # [cos;sin]
    qcs = nc.dram_tensor("qcs", [64, 1024], BF16, kind="ExternalInput")
    masks = nc.dram_tensor("masks", [4, 128, 256], BF16, kind="ExternalInput")
    out = nc.dram_tensor("out", [D, 1024], BF16, kind="ExternalOutput")

    # swap matrix: exchanges 32-partition blocks 0<->1, 2<->3 (rope pairs)
    SW = np.zeros((128, 128), ml_dtypes.bfloat16)
    for blk in range(4):
        srcb = blk ^ 1
        for i in range(32):
            SW[blk * 32 + i, srcb * 32 + i] = 1.0
    sw_dram = nc.inline_tensor(SW, name="swconst")
    # full-half exchange: out partition p <- in partition (p+64)%128
    XC = np.zeros((128, 128), ml_dtypes.bfloat16)
    for p in range(128):
        XC[(p + 64) % 128, p] = 1.0
    xc_dram = nc.inline_tensor(XC, name="xchconst")
    sign = np.zeros((128, 1), np.float32)
    for blk in range(4):
        sign[blk * 32:(blk + 1) * 32] = -1.0 if blk % 2 == 0 else 1.0
    sign_dram = nc.inline_tensor(sign, name="signconst")
    id_dram = nc.inline_tensor(np.eye(128, dtype=ml_dtypes.bfloat16),
                               name="idconst")

    def mm(out_ap, lhsT, rhs, start, stop, tile_position=None):
        nc.tensor.matmul(
            out_ap, lhsT, rhs,
            start=start, stop=stop,
            tile_position=tile_position,
            skip_group_check=True,
        )

    def bcast4(engine, dst, src_dram, row0, col0, width, nrow=32):
        # replicate [nrow, width] of src 128//nrow times across partitions
        base = src_dram[row0:row0 + nrow, col0:col0 + width]
        ap = bass.AP(
            tensor=base.tensor,
            offset=base.offset,
            ap=[[0, 128 // nrow]] + list(base.ap),
        )
        engine.dma_start(out=dst, in_=ap)

    def rep4(base):
        # view a [128, 256] tile as [128, 1024] by repeating the free dim 4x
        return bass.AP(
            tensor=base.tensor,
            offset=base.offset,
            ap=[list(base.ap)[0], [0, 4], list(base.ap)[1]],
        )

    with tile.TileContext(nc) as tc:
        with tc.tile_pool(name="persist", bufs=1) as persist:
            qT = persist.tile([64, H, 1024], BF16)
            kT = persist.tile([64, KVH, S], BF16)
            V = persist.tile([128, 16, 256], BF16)
            wk_sb = persist.tile([128, 6, 256], BF16)
            wv_sb = persist.tile([128, 6, 256], BF16)
            sw_sb = persist.tile([128, 128], BF16)
            xc_sb = persist.tile([128, 128], BF16)
            id_sb = persist.tile([128, 128], BF16)
            sign_sb = persist.tile([128, 1], F32)
            ones64 = persist.tile([128, 64], BF16)
            cosk = persist.tile([128, S], BF16)
            sink = persist.tile([128, S], BF16)
            cosq = persist.tile([128, 1024], BF16)
            sinq = persist.tile([128, 1024], BF16)
            masks_sb = persist.tile([128, 4, 256 if use_rep4 else 1024],
                                    BF16)
            wo_sb = persist.tile([128, 6, D], BF16)
            attnT = persist.tile([128, 6, 1024], BF16)

            # ---- input DMA issue: EDF round-robin over engine queues ----
            # sync has 4 hw queues, scalar 4, gpsimd 8 (software DGE); each
            # queue serializes its transfers, so spread by deadline.
            nc.sync.dma_start(out=id_sb[:, :], in_=id_dram[:, :])
            nc.sync.dma_start(out=sw_sb[:, :], in_=sw_dram[:, :])
            nc.sync.dma_start(out=xc_sb[:, :], in_=xc_dram[:, :])
            nc.sync.dma_start(out=sign_sb[:, :], in_=sign_dram[:, :])
            # gpsimd round 1: first k rope table chunks (needed from ~8us)
            for c in range(2):
                bcast4(nc.gpsimd, cosk[:, c * 512:(c + 1) * 512], kcs, 0,
                       c * 512, 512)
                bcast4(nc.gpsimd, sink[:, c * 512:(c + 1) * 512], kcs, 32,
                       c * 512, 512)
            # scalar round 1: wk bundled + first xT tail chunks + masks
            src = wk[:, :]
            nc.scalar.dma_start(
                out=wk_sb[:, :, :],
                in_=bass.AP(tensor=src.tensor, offset=src.offset,
                            ap=[[256, 128], [256 * 128, 6], [1, 256]]))
            src = wv[:, :]
            nc.scalar.dma_start(
                out=wv_sb[:, :, :],
                in_=bass.AP(tensor=src.tensor, offset=src.offset,
                            ap=[[256, 128], [256 * 128, 6], [1, 256]]))
            nc.vector.memset(ones64[:, :], 1.0)

            def load_masks():
                if use_rep4:
                    src = masks[:, :, :]
                    nc.sync.dma_start(
                        out=masks_sb[:, :, :],
                        in_=bass.AP(tensor=src.tensor, offset=src.offset,
                                    ap=[[256, 128], [128 * 256, 4],
                                        [1, 256]]))
                else:
                    for i in range(4):
                        base = masks[i, :, :]
                        rep = bass.AP(tensor=base.tensor, offset=base.offset,
                                      ap=[list(base.ap)[0], [0, 4],
                                          list(base.ap)[1]])
                        nc.sync.dma_start(out=masks_sb[:, i, :], in_=rep)

            # ---------------- Phase 1: projections + rope ----------------
            with tc.tile_pool(name="p1", bufs=1) as p1, \
                 tc.tile_pool(name="tmp", bufs=6) as tmpp, \
                 tc.tile_pool(name="psA", bufs=4, space="PSUM") as psA, \
                 tc.tile_pool(name="psB", bufs=2, space="PSUM") as psB:

                xT_sb = p1.tile([128, 6, S], BF16)
                xqT_sb = p1.tile([128, 6, 1024], BF16)
                wq_sb = p1.tile([128, 6, H * HD], BF16)

                def wq_piece(dt, engine):
                    engine.dma_start(out=wq_sb[:, dt, :],
                                     in_=wq[dt * 128:(dt + 1) * 128, :])

                # sync: xT column chunks in consumption order (4 queues),
                # with two wq pieces slotted between c1 and c2
                for c in range(4):
                    if c == 2:
                        wq_piece(0, nc.sync)
                        wq_piece(1, nc.sync)
                    for dt in range(4):
                        nc.sync.dma_start(
                            out=xT_sb[:, dt, c * 512:(c + 1) * 512],
                            in_=xT[dt * 128:(dt + 1) * 128,
                                   c * 512:(c + 1) * 512])
                def xq_quarter(dthalf, chalf, eng=None):
                    # [128, 3 dt, 512] bundle of xqT
                    d0 = 3 * dthalf
                    base = xqT[d0 * 128:(d0 + 3) * 128,
                               chalf * 512:(chalf + 1) * 512]
                    (eng or nc.scalar).dma_start(
                        out=xqT_sb[:, d0:d0 + 3,
                                   chalf * 512:(chalf + 1) * 512],
                        in_=bass.AP(tensor=base.tensor, offset=base.offset,
                                    ap=[[1024, 128], [1024 * 128, 3],
                                        [1, 512]]))

                # scalar first-wave ONLY (one DMA per chain, issued with
                # no chain-predecessor wait): anything more on the scalar
                # queue would block all ACT compute behind its flow-control
                # waits. The rest goes to sync and gpsimd, which run no
                # compute.
                xq_quarter(0, 0)
                xq_quarter(1, 0)
                # sync (continued): xqT second half + masks
                xq_quarter(0, 1, nc.sync)
                xq_quarter(1, 1, nc.sync)
                load_masks()
                # gpsimd: xT dt4/5 chunks, rope tables, wq pieces, wo
                def x_tail2(c, eng):
                    for dt in (4, 5):
                        eng.dma_start(
                            out=xT_sb[:, dt, c * 512:(c + 1) * 512],
                            in_=xT[dt * 128:(dt + 1) * 128,
                                   c * 512:(c + 1) * 512])

                x_tail2(0, nc.gpsimd)
                bcast4(nc.gpsimd, cosq[:, :], qcs, 0, 0, 1024)
                bcast4(nc.gpsimd, sinq[:, :], qcs, 32, 0, 1024)
                x_tail2(1, nc.gpsimd)
                wq_piece(2, nc.gpsimd)
                wq_piece(3, nc.gpsimd)
                wq_piece(4, nc.gpsimd)
                wq_piece(5, nc.gpsimd)
                x_tail2(2, nc.gpsimd)
                for c in range(2, 4):
                    bcast4(nc.gpsimd, cosk[:, c * 512:(c + 1) * 512], kcs, 0,
                           c * 512, 512)
                    bcast4(nc.gpsimd, sink[:, c * 512:(c + 1) * 512], kcs, 32,
                           c * 512, 512)
                x_tail2(3, nc.gpsimd)
                for hd in range(6):
                    nc.gpsimd.dma_start(out=wo_sb[:, hd, :],
                                        in_=wo[hd * 128:(hd + 1) * 128, :])

                # PE warmup: ramps the p-state while DMAs land, and preloads
                # the Exp activation table.
                warm_ps = psA.tile([128, 512], F32, tag="pj")
                for wi in range(36):
                    mm(warm_ps[:, 0:128], id_sb[:, :], id_sb[:, :],
                       start=(wi == 0), stop=(wi == 35))
                warm_sb = tmpp.tile([128, 512], F32, tag="ta")
                nc.scalar.activation(
                    out=warm_sb[:, 0:128], in_=warm_ps[:, 0:128],
                    func=mybir.ActivationFunctionType.Exp, scale=0.01)

                def rope_pair(proj_ps, cos_t, sin_t):
                    # rope on a [128, 512] pair tile; returns (sw_ps, ta):
                    # result = sign * sw_ps + ta, to be combined per-half
                    ta = tmpp.tile([128, 512], F32, tag="ta")
                    tb = tmpp.tile([128, 512], BF16, tag="tb")
                    nc.vector.tensor_mul(ta[:, :], proj_ps, cos_t)
                    nc.vector.tensor_mul(tb[:, :], proj_ps, sin_t)
                    sw_ps = psB.tile([128, 512], F32, tag="b")
                    mm(sw_ps[:, :], sw_sb[:, :], tb[:, :], start=True,
                       stop=True)
                    return sw_ps, ta

                def stt_half(dst, sw_ps, ta, p0):
                    nc.vector.scalar_tensor_tensor(
                        out=dst,
                        in0=sw_ps[p0:p0 + 64, :],
                        scalar=sign_sb[p0:p0 + 64, 0:1],
                        in1=ta[p0:p0 + 64, :],
                        op0=mybir.AluOpType.mult,
                        op1=mybir.AluOpType.add,
                    )

                def rope_store(sw_ps, ta, cs, dst_even, dst_odd):
                    # combine rope halves into ro, then land both heads at
                    # partition base 0: even half is an aligned copy; odd half
                    # goes through the half-exchange matmul first (base-64
                    # matmul OPERANDS hang this runtime, so weights/rhs must
                    # stay at partition 0 — the exchange runs on data instead)
                    ro = tmpp.tile([128, 512], BF16, tag="ro")
                    stt_half(ro[0:64, :], sw_ps, ta, 0)
                    stt_half(ro[64:128, :], sw_ps, ta, 64)
                    nc.scalar.copy(dst_even, ro[0:64, :])
                    fs_ps = psB.tile([128, 512], F32, tag="b")
                    mm(fs_ps[:, :], xc_sb[:, :], ro[:, :], start=True,
                       stop=True)
                    nc.scalar.copy(dst_odd, fs_ps[0:64, :])

                def k_chunk(c):
                    cs = slice(c * 512, (c + 1) * 512)
                    for m in range(2):
                        k_ps = psA.tile([128, 512], F32, tag="pj")
                        for dt in range(6):
                            mm(k_ps[:, :], wk_sb[:, dt, m * 128:(m + 1) * 128],
                               xT_sb[:, dt, cs],
                               start=(dt == 0), stop=(dt == 5))
                        sw_ps, ta = rope_pair(k_ps[:, :], cosk[:, cs],
                                              sink[:, cs])
                        rope_store(sw_ps, ta, cs,
                                   kT[:, 2 * m, cs], kT[:, 2 * m + 1, cs])

                def q_chunk(c):
                    cs = slice(c * 512, (c + 1) * 512)
                    for t in range(6):
                        q_ps = psA.tile([128, 512], F32, tag="pj")
                        for dt in range(6):
                            mm(q_ps[:, :], wq_sb[:, dt, t * 128:(t + 1) * 128],
                               xqT_sb[:, dt, cs],
                               start=(dt == 0), stop=(dt == 5))
                        sw_ps, ta = rope_pair(q_ps[:, :], cosq[:, cs],
                                              sinq[:, cs])
                        rope_store(sw_ps, ta, cs,
                                   qT[:, 2 * t, cs], qT[:, 2 * t + 1, cs])

                def v_tiles(c):
                    for st in range(4 * c, 4 * c + 4):
                        v_ps = psA.tile([128, 256], F32, tag="pj")
                        for dt in range(6):
                            mm(v_ps[:, :],
                               xT_sb[:, dt, st * 128:(st + 1) * 128],
                               wv_sb[:, dt, :], start=(dt == 0), stop=(dt == 5))
                        nc.scalar.copy(V[:, st, :], v_ps[:, :])

                k_chunk(0)
                v_tiles(0)
                k_chunk(1)
                v_tiles(1)
                k_chunk(2)
                v_tiles(2)
                q_chunk(0)
                k_chunk(3)
                v_tiles(3)
                q_chunk(1)

            if phases == 1:
                for t in range(6):
                    nc.sync.dma_start(
                        out=out[t * 128:t * 128 + 64, :],
                        in_=qT[:, 2 * t, :])
                    nc.sync.dma_start(
                        out=out[t * 128 + 64:(t + 1) * 128, :],
                        in_=qT[:, 2 * t + 1, :])

            # ---------------- Phase 2: attention + wo ----------------
            if phases >= 2:
              with tc.tile_pool(name="expp", bufs=3) as expp, \
                   tc.tile_pool(name="dacc", bufs=2) as dacc, \
                   tc.tile_pool(name="nrm", bufs=4) as nrm, \
                   tc.tile_pool(name="zsb", bufs=3) as zsb, \
                   tc.tile_pool(name="psSC", bufs=2, space="PSUM") as psSC, \
                   tc.tile_pool(name="psPV", bufs=2, space="PSUM") as psPV, \
                   tc.tile_pool(name="psDN", bufs=1, space="PSUM") as psDN, \
                   tc.tile_pool(name="psZ", bufs=1, space="PSUM") as psZ:

                def wo_mi(sq, mi, pool):
                    # one 128-row wo chunk for query chunk sq (256 cols)
                    z_ps = pool.tile([128, 256], F32, tag="dn" if pool is psDN else "z")
                    for hd in range(6):
                        mm(z_ps[:, :],
                           wo_sb[:, hd, mi * 128:(mi + 1) * 128],
                           attnT[:, hd, sq * 256:(sq + 1) * 256],
                           start=(hd == 0), stop=(hd == 5))
                    z_sb = zsb.tile([128, 256], BF16, tag="z")
                    nc.scalar.copy(z_sb[:, :], z_ps[:, :])
                    nc.sync.dma_start(
                        out=out[mi * 128:(mi + 1) * 128,
                                sq * 256:(sq + 1) * 256],
                        in_=z_sb[:, :])

                def emit_pv(grp, pv_ps, tgt, kt, n_kt):
                    for pi, p in enumerate(grp):
                        for half in range(2):
                            h = PAIRS[p][half]
                            kv = _kv(h)
                            obase = (h % 2) * 64
                            o = pi * 512 + half * 256
                            mm(pv_ps[obase:obase + 64, pi, :],
                               V[:, kt, kv * 64:(kv + 1) * 64],
                               tgt[:, o:o + 256],
                               start=(kt == 0 and pi == 0),
                               stop=(kt == n_kt - 1),
                               tile_position=(0, obase))

                def emit_den(grp, den_ps, src, first, stop):
                    # four 256-col ones-matmul streams accumulating per-head
                    # denominators into den_ps (partition-matched layout)
                    for pi, p in enumerate(grp):
                        for half in range(2):
                            obase = half * 64
                            o = pi * 512 + half * 256
                            mm(den_ps[obase:obase + 64, pi, :],
                               ones64[:, :], src[:, o:o + 256],
                               start=(first and pi == 0), stop=stop,
                               tile_position=(0, obase))

                # den-PE kts: enough per-kt PE work to keep the engine
                # wait-free (it locks to a low p-state otherwise); the rest
                # accumulate on the Vector engine into acc
                DEN_PE = {
                    0: {0, 1, 2, 3},
                    1: {4, 5, 6, 7},
                    2: {2, 8, 9, 10, 11},
                    3: {2, 3, 4, 5, 12, 13, 14, 15},
                }

                def make_finalize(s, grp, acc, pv_ps, den_ps, last_tgt, n_kt):
                    has_fold = len(DEN_PE[s]) < n_kt
                    def fin():
                        # flush the software-pipelined last PV + den
                        emit_pv(grp, pv_ps, last_tgt, n_kt - 1, n_kt)
                        emit_den(grp, den_ps, last_tgt,
                                 first=(DEN_PE[s] == {n_kt - 1}),
                                 stop=not has_fold)
                        if has_fold:
                            emit_den(grp, den_ps, acc, first=False, stop=True)
                        rec = nrm.tile([128, 2, 256], F32, tag="rec")
                        nc.vector.reciprocal_approx_fast(
                            rec[:, :, :], den_ps[:, :, :])
                        for pi, p in enumerate(grp):
                            nc.vector.tensor_mul(
                                attnT[:, p, s * 256:(s + 1) * 256],
                                pv_ps[:, pi, :],
                                rec[:, pi, :])
                    return fin

                pending = None
                prev_s = None
                for s in (3, 2, 1, 0):
                    # s=3 first: its deep kt chains keep the PE saturated
                    # through the projection->attention transition so the
                    # clock ramp is paid once
                    n_kt = 4 * s + 4
                    # wo chunks of the previously processed s spread evenly
                    # over this s's kt slots
                    n_slot = 3 * n_kt
                    wo_slots = {}
                    if phases >= 3 and prev_s is not None:
                        for mi in range(6):
                            wo_slots[((2 * mi + 1) * n_slot) // 12] = mi
                    den_pe = DEN_PE[s]

                    for gi, grp in enumerate(GROUPS):
                        acc = dacc.tile([128, 1024], BF16, tag="acc")
                        pv_ps = psPV.tile([128, 2, 256], F32)
                        den_ps = psDN.tile([128, 2, 256], F32, tag="dn")
                        den_first = True
                        prev_tgt = None
                        for kt in range(n_kt):
                            slot = gi * n_kt + kt
                            sc_ps = psSC.tile([128, 1024], F32)
                            # scores (transposed): [k 128, q 256] per head
                            for pi, p in enumerate(grp):
                                hA, hB = PAIRS[p]
                                o = pi * 512
                                if _kv(hA) == _kv(hB):
                                    mm(sc_ps[:, o:o + 512],
                                       kT[:, _kv(hA),
                                          kt * 128:(kt + 1) * 128],
                                       qT[:, hA:hA + 2,
                                          s * 256:(s + 1) * 256],
                                       start=True, stop=True)
                                else:
                                    for half, h in enumerate((hA, hB)):
                                        mm(sc_ps[:, o + half * 256:
                                                 o + half * 256 + 256],
                                           kT[:, _kv(h),
                                              kt * 128:(kt + 1) * 128],
                                           qT[:, h, s * 256:(s + 1) * 256],
                                           start=(half == 0), stop=True)
                            tgt = expp.tile([128, 1024], BF16, tag="e")
                            nc.scalar.activation(
                                out=tgt[:, :],
                                in_=sc_ps[:, :],
                                func=mybir.ActivationFunctionType.Exp,
                                scale=0.125,
                            )
                            if kt >= 4 * s:
                                mk = masks_sb[:, kt - 4 * s, :]
                                nc.vector.tensor_mul(
                                    tgt[:, :], tgt[:, :],
                                    rep4(mk) if use_rep4 else mk)
                            # finalize previous grp between this grp's first
                            # scores and its first PV matmuls
                            if kt == 0 and pending is not None:
                                pending()
                                pending = None
                            if kt not in den_pe:
                                if kt == 1:
                                    # first accumulation combines kt0+kt1
                                    # (kt0's exp can't live in acc: the
                                    # pipelined PV of kt0 still reads it)
                                    nc.vector.tensor_add(
                                        acc[:, :], prev_tgt[:, :], tgt[:, :])
                                elif kt > 1:
                                    nc.vector.tensor_add(
                                        acc[:, :], acc[:, :], tgt[:, :])
                            # software pipelining: PV (+den) of the PREVIOUS
                            # kt, so the in-order PE never blocks on this
                            # kt's exp->mask chain
                            if prev_tgt is not None:
                                emit_pv(grp, pv_ps, prev_tgt, kt - 1, n_kt)
                                if kt - 1 in den_pe:
                                    emit_den(grp, den_ps, prev_tgt,
                                             first=den_first, stop=False)
                                    den_first = False
                            prev_tgt = tgt
                            if slot in wo_slots:
                                wo_mi(prev_s, wo_slots[slot], psZ)
                        pending = make_finalize(s, grp, acc, pv_ps, den_ps,
                                                prev_tgt, n_kt)
                    prev_s = s

                if pending is not None:
                    pending()
                    pending = None
                if phases >= 3:
                    # tail: wo of the last processed s, alternating psum
                    # pools so the PE never waits on the drain copies
                    for mi in range(6):
                        wo_mi(prev_s, mi, psZ if mi % 2 == 0 else psDN)

                if phases == 2:
                    for t in range(6):
                        nc.sync.dma_start(
                            out=out[t * 128:(t + 1) * 128, :],
                            in_=attnT[:, t, :])

    nc.compile()
    return nc


# ---------------------------------------------------------------------------
# host side
# ---------------------------------------------------------------------------

def _permute_cols(w, nheads):
    """Deinterleave rope pairs within each head: even dims then odd dims."""
    cols = []
    for h in range(nheads):
        blk = w[:, h * HD:(h + 1) * HD]
        cols.append(blk[:, 0::2])
        cols.append(blk[:, 1::2])
    return np.ascontiguousarray(np.concatenate(cols, axis=1))


def make_in_maps(x, wq, wk, wv, wo, freqs_cos, freqs_sin):
    bf = ml_dtypes.bfloat16
    wq_p = _permute_cols(np.asarray(wq, np.float32), H).astype(bf)
    wk_p = _permute_cols(np.asarray(wk, np.float32), KVH).astype(bf)
    wv_b = np.ascontiguousarray(np.asarray(wv, np.float32)).astype(bf)
    wo_b = np.ascontiguousarray(np.asarray(wo, np.float32)).astype(bf)

    cosT = np.ascontiguousarray(np.asarray(freqs_cos, np.float32).T)  # [32, S]
    sinT = np.ascontiguousarray(np.asarray(freqs_sin, np.float32).T)
    kcs = np.ascontiguousarray(np.concatenate([cosT, sinT], axis=0))  # [64, S]

    in_maps = []
    for core in range(8):
        b, role = core // 2, core % 2
        xT = np.ascontiguousarray(np.asarray(x[b], np.float32).T)
        q_rows = np.concatenate(
            [np.arange(256 * (2 * s + role), 256 * (2 * s + role) + 256)
             for s in range(4)])
        xqT = np.ascontiguousarray(xT[:, q_rows])
        qcs = np.ascontiguousarray(kcs[:, q_rows])
        # only 4 distinct masks: diagonal offset 256*role - 128*rr
        m = np.zeros((4, 128, 256), np.float32)
        ar = np.arange(256)[None, :]
        pr = np.arange(128)[:, None]
        for rr in range(4):
            m[rr] = (pr <= ar + 256 * role - 128 * rr).astype(np.float32)
        in_maps.append({
            "xT": xT.astype(bf),
            "xqT": xqT.astype(bf),
            "wq": wq_p,
            "wk": wk_p,
            "wv": wv_b,
            "wo": wo_b,
            "kcs": kcs.astype(bf),
            "qcs": qcs.astype(bf),
            "masks": m.astype(bf),
        })
    return in_maps


_NC_CACHE = {}


def kernel(x, wq, wk, wv, wo, freqs_cos, freqs_sin, mask_attention,
           start_pos=0, inference=0, **_ignored):
    from concourse.bass_utils import run_bass_kernel_spmd

    in_maps = make_in_maps(np.asarray(x, np.float32), wq, wk, wv, wo,
                           freqs_cos, freqs_sin)
    if "nc" not in _NC_CACHE:
        _NC_CACHE["nc"] = build_nc()
    nc = _NC_CACHE["nc"]
    res = run_bass_kernel_spmd(nc, in_maps, core_ids=list(range(8)))
    outs = res.results
    out_full = np.zeros((B, S, D), np.float32)
    for core in range(8):
        b, role = core // 2, core % 2
        zT = np.asarray(outs[core]["out"], np.float32)  # [768, 1024]
        for s in range(4):
            j = 2 * s + role
            out_full[b, 256 * j:256 * j + 256, :] = zT[:, 256 * s:256 * s + 256].T
    return out_full
